# revision 1
# baseline (speedup 1.0000x reference)
"""Trainium2 Bass kernel for the DigitConvolutionalModel problem.

Math: out = relu(conv3x3(x) @ fc1_w.T + fc1_b) @ fc2_w.T + fc2_b
The 3x3 valid conv followed by a dense layer composes into a single
linear map, so conv_w and fc1_w are folded on the host into one
W1eff [128, 784] matrix. The device then runs two matmuls + bias/relu.

Sharding: pure data parallelism — batch split across 8 cores.
Each core's x shard is staged transposed ([784, 8192]) so the
contraction dim lands on SBUF partitions with contiguous DMA.

Precision: fc1 runs as a compensated fp16 product — x and W1eff are
each split into hi+lo fp16 pairs (same total bytes over HBM as f32)
and combined as xh@Wh + xh@Wl + xl@Wh into the f32 PSUM, giving
near-f32 accuracy at fp16 matmul throughput. The three 16-row K-tail
products are packed into one 48-row chunk so every matmul contracts
a full-ish partition block. fc2 (tiny K=128) runs in plain f32.
"""

import numpy as np

import concourse.bacc as bacc
import concourse.mybir as mybir
import concourse.tile as tile
from concourse.bass_utils import run_bass_kernel_spmd

N_CORES = 8
B = 65536
B_LOCAL = B // N_CORES  # 8192
K = 784                 # input features (28*28)
KM = 768                # main K rows (6 chunks of 128)
KT = 48                 # packed tail rows: [xh_t; xh_t; xl_t] x 16
M1 = 128                # fc1 out
M2 = 10                 # fc2 out
NKC = 6                 # main K chunks

F32 = mybir.dt.float32
FP16 = mybir.dt.float16

MODE = "fp16x2"
BT = 2048               # batch tile per DMA
NS = 512                # matmul moving-dim subtile (one PSUM bank)

_cache = {}


def _bt_schedule(total=B_LOCAL, ns=NS, bt=1024):
    """Uniform tiles: DMA delivery and PE consumption rates are nearly
    equal, so any size jump starves one side."""
    assert total % bt == 0 and bt % ns == 0
    return [bt] * (total // bt)


def _build_nc(mode=MODE, bt=BT, ns=NS):
    assert mode == "fp16x2"
    nc = bacc.Bacc("TRN2", target_bir_lowering=False, debug=False,
                   num_devices=N_CORES)

    xh_d = nc.dram_tensor("x_h", [KM, B_LOCAL], FP16, kind="ExternalInput")
    xl_d = nc.dram_tensor("x_l", [KM, B_LOCAL], FP16, kind="ExternalInput")
    xt_d = nc.dram_tensor("x_tail", [KT, B_LOCAL], FP16, kind="ExternalInput")
    # All matmul weights packed as column blocks of one [128, 1684] tensor:
    # cols 0:768 = 6 Wh chunks, 768:1536 = 6 Wl chunks, 1536:1664 = packed
    # tail (rows 0:48), 1664:1674 = W2h, 1674:1684 = W2l.
    wall_d = nc.dram_tensor("w_all", [128, 1664], FP16, kind="ExternalInput")
    # f32 pack: col 0 = b1, col 1 rows 0:10 = b2, cols 2:12 = W2 (f32)
    bias_d = nc.dram_tensor("biases", [M1, 12], F32, kind="ExternalInput")
    z_d = nc.dram_tensor("z_t", [M2, B_LOCAL], F32, kind="ExternalOutput")

    with tile.TileContext(nc) as tc:
        with (
            tc.tile_pool(name="static", bufs=1) as sp,
            tc.tile_pool(name="xp", bufs=4) as xp,
            tc.tile_pool(name="hp", bufs=8) as hp,
            tc.tile_pool(name="zp", bufs=3) as zp,
            tc.tile_pool(name="pp1", bufs=4, space="PSUM") as pp1,
            tc.tile_pool(name="pp2", bufs=2, space="PSUM") as pp2,
        ):
            # One DMA for all weights, one for both biases, on the
            # (otherwise idle) GPSIMD SWDGE path — off the HWDGE x rings.
            wall = sp.tile([128, 1664], FP16, tag="w_all")
            nc.gpsimd.dma_start(wall[:], wall_d[:])
            w1hs = [wall[:, kc * 128:(kc + 1) * 128] for kc in range(NKC)]
            w1ls = [wall[:, 768 + kc * 128: 768 + (kc + 1) * 128]
                    for kc in range(NKC)]
            wtl = wall[0:KT, 1536:1664]

            bts = _bt_schedule(B_LOCAL, ns)
            offs = [sum(bts[:i]) for i in range(len(bts))]
            xtiles = [None] * len(bts)
            # [768, B] viewed as [128 partitions, 6 chunks, B] so one SWDGE
            # DMA moves all six k-chunks of a batch tile.
            xh_v = xh_d.rearrange("(c p) b -> p c b", p=128)
            xl_v = xl_d.rearrange("(c p) b -> p c b", p=128)

            def load_bt(i):
                """Issue bt i's x DMAs (3 fused SWDGE transfers)."""
                btc = bts[i]
                bsl = slice(offs[i], offs[i] + btc)
                xh_all = xp.tile([128, NKC, btc], FP16, tag="xh")
                nc.gpsimd.dma_start(xh_all[:], xh_v[:, :, bsl])
                xl_all = xp.tile([128, NKC, btc], FP16, tag="xl")
                nc.gpsimd.dma_start(xl_all[:], xl_v[:, :, bsl])
                xtl = xp.tile([KT, btc], FP16, tag="xtail")
                nc.gpsimd.dma_start(xtl[:], xt_d[:, bsl])
                xhs = [xh_all[:, kc, :] for kc in range(NKC)]
                xls = [xl_all[:, kc, :] for kc in range(NKC)]
                xtiles[i] = (xhs, xls, xtl)

            # bt0 is the pipeline fill: load it as interleaved half-chunk
            # tiles (xh chunks 0-2, xl 0-2, xh 3-5, xl 3-5) and reorder the
            # accumulation so the PE starts ~4us sooner and never waits a
            # full 4.4us transfer mid-chain. Bias rides behind the first x.
            bt0 = bts[0]
            xh0a = sp.tile([128, 3, bt0], FP16, tag="xh0a")
            nc.gpsimd.dma_start(xh0a[:], xh_v[:, 0:3, 0:bt0])
            xl0a = sp.tile([128, 3, bt0], FP16, tag="xl0a")
            nc.gpsimd.dma_start(xl0a[:], xl_v[:, 0:3, 0:bt0])
            xh0b = sp.tile([128, 3, bt0], FP16, tag="xh0b")
            nc.gpsimd.dma_start(xh0b[:], xh_v[:, 3:6, 0:bt0])
            xl0b = sp.tile([128, 3, bt0], FP16, tag="xl0b")
            nc.gpsimd.dma_start(xl0b[:], xl_v[:, 3:6, 0:bt0])
            bias = sp.tile([M1, 12], F32, tag="biases")
            nc.gpsimd.dma_start(bias[:], bias_d[:])
            xtl0 = sp.tile([KT, bt0], FP16, tag="xtail0")
            nc.gpsimd.dma_start(xtl0[:], xt_d[:, 0:bt0])
            b1t = bias[:, 0:1]
            b2t = bias[0:M2, 1:2]
            w2t = bias[:, 2:12]
            xtiles[0] = (
                [xh0a[:, c, :] for c in range(3)]
                + [xh0b[:, c, :] for c in range(3)],
                [xl0a[:, c, :] for c in range(3)]
                + [xl0b[:, c, :] for c in range(3)],
                xtl0,
            )
            # bt0 pair order matches delivery: (xhA passes, xlA pass,
            # xhB passes, xlB pass, tail)
            bt0_pairs_idx = (
                [("h", kc) for kc in range(3)] + [("l", kc) for kc in range(3)]
                + [("x", kc) for kc in range(3)]
                + [("h", kc) for kc in range(3, 6)]
                + [("l", kc) for kc in range(3, 6)]
                + [("x", kc) for kc in range(3, 6)]
            )
            load_bt(1)
            load_bt(2)
            # Each chain's fc2 matmul is deferred until after the NEXT
            # chain's fc1 stream, so the PE never waits on ACT's h output.
            pending = []

            zq = []

            def flush_pending():
                for h_t, zt_t, sl_t, final in pending:
                    ps2 = pp2.tile([M2, ns], F32, tag="ps2")
                    nc.tensor.matmul(
                        ps2[:], w2t, h_t[:], start=True, stop=True)
                    nc.vector.tensor_scalar_add(zt_t[:, sl_t], ps2[:], b2t)
                    if final is not None:
                        zq.append((final[0], zt_t[:]))
                pending.clear()

            for bt_i, btc in enumerate(bts):
                if bt_i + 3 < len(bts):
                    load_bt(bt_i + 3)  # prefetch three batch tiles ahead
                if len(zq) >= 2:
                    nc.gpsimd.dma_start(*zq.pop(0))
                bsl = slice(offs[bt_i], offs[bt_i] + btc)
                xhs, xls, xtl = xtiles[bt_i]
                zt = zp.tile([M2, btc], F32, tag="z")
                nchains = btc // ns
                for ns_i in range(nchains):
                    sl = slice(ns_i * ns, (ns_i + 1) * ns)
                    ps1 = pp1.tile([M1, ns], F32, tag="ps1")
                    if bt_i == 0:
                        sel = {"h": (w1hs, xhs), "l": (w1ls, xhs),
                               "x": (w1hs, xls)}
                        pairs = [(sel[p][0][kc], sel[p][1][kc])
                                 for p, kc in bt0_pairs_idx] + [(wtl, xtl)]
                    else:
                        pairs = (
                            [(w1hs[kc], xhs[kc]) for kc in range(NKC)]
                            + [(w1ls[kc], xhs[kc]) for kc in range(NKC)]
                            + [(w1hs[kc], xls[kc]) for kc in range(NKC)]
                            + [(wtl, xtl)]
                        )
                    for i, (wt, xt) in enumerate(pairs):
                        nc.tensor.matmul(
                            ps1[:], wt, xt[:, sl],
                            start=(i == 0), stop=(i == len(pairs) - 1))
                    h = hp.tile([M1, ns], F32, tag="h")
                    nc.scalar.activation(
                        h[:], ps1[:], mybir.ActivationFunctionType.Relu,
                        bias=b1t)
                    flush_pending()
                    final = (z_d[:, bsl],) if ns_i == nchains - 1 else None
                    pending.append((h, zt, sl, final))
            flush_pending()
            for args in zq:
                nc.gpsimd.dma_start(*args)
    nc.compile()
    return nc


def _fold_weights(conv_w, fc1_w):
    """Fold 3x3 valid cross-correlation + fc1 into one [128, 784] matrix."""
    cw = np.asarray(conv_w, np.float64)
    f1 = np.asarray(fc1_w, np.float64).reshape(M1, 26, 26)
    W = np.zeros((M1, 28, 28), np.float64)
    for di in range(3):
        for dj in range(3):
            W[:, di:di + 26, dj:dj + 26] += cw[di, dj] * f1
    return W.reshape(M1, K).astype(np.float32)


def _split16(a):
    hi = a.astype(np.float16)
    lo = (a.astype(np.float32) - hi.astype(np.float32)).astype(np.float16)
    return hi, lo


def kernel(x, conv_w, fc1_w, fc1_b, fc2_w, fc2_b):
    if "nc" not in _cache:
        _cache["nc"] = _build_nc()
    nc = _cache["nc"]

    w1t = np.ascontiguousarray(_fold_weights(conv_w, fc1_w).T)  # [784, 128]
    w1t_h, w1t_l = _split16(w1t)
    w_tail = np.vstack([w1t_h[KM:], w1t_l[KM:], w1t_h[KM:]])  # [48, 128]
    w2t = np.asarray(fc2_w, np.float32).T  # [128, 10]
    w_all = np.zeros((128, 1664), np.float16)
    for kc in range(NKC):
        w_all[:, kc * 128:(kc + 1) * 128] = w1t_h[kc * 128:(kc + 1) * 128, :]
        w_all[:, 768 + kc * 128: 768 + (kc + 1) * 128] = \
            w1t_l[kc * 128:(kc + 1) * 128, :]
    w_all[0:KT, 1536:1664] = w_tail
    w_all = np.ascontiguousarray(w_all)
    biases = np.zeros((M1, 12), np.float32)
    biases[:, 0] = np.asarray(fc1_b, np.float32)
    biases[0:M2, 1] = np.asarray(fc2_b, np.float32)
    biases[:, 2:12] = w2t
    x = np.asarray(x, np.float32)

    in_maps = []
    for c in range(N_CORES):
        xs = np.ascontiguousarray(x[c * B_LOCAL:(c + 1) * B_LOCAL].T)
        xh, xl = _split16(xs)
        # tail rows ordered to match w_tail: [xh_t (vs Wh), xh_t (vs Wl),
        # xl_t (vs Wh)]
        x_tail = np.ascontiguousarray(
            np.vstack([xh[KM:], xh[KM:], xl[KM:]]))  # [48, B_LOCAL]
        in_maps.append({
            "x_h": np.ascontiguousarray(xh[:KM]),
            "x_l": np.ascontiguousarray(xl[:KM]),
            "x_tail": x_tail,
            "w_all": w_all, "biases": biases,
        })
    res = run_bass_kernel_spmd(nc, in_maps, list(range(N_CORES)))
    outs = [res.results[c]["z_t"].T for c in range(N_CORES)]
    return np.ascontiguousarray(np.concatenate(outs, axis=0), dtype=np.float32)



# revision 3
# speedup vs baseline: 1.9274x; 1.9274x over previous
"""Trainium2 Bass kernel for the DigitConvolutionalModel problem.

Math: out = relu(conv3x3(x) @ fc1_w.T + fc1_b) @ fc2_w.T + fc2_b
The 3x3 valid conv followed by a dense layer composes into a single
linear map, so conv_w and fc1_w are folded on the host into one
W1eff [128, 784] matrix. The device then runs two matmuls + bias/relu.

Sharding: pure data parallelism - batch split across 8 cores.
Each core's x shard is staged transposed ([784, 8192]) so the
contraction dim lands on SBUF partitions with contiguous DMA.

Precision: plain fp16 for x and all weights with f32 PSUM
accumulation. The correctness budget (rel err 2e-2) dwarfs fp16
matmul error (~1e-3 here), and fp16 halves HBM traffic vs f32 or a
compensated hi+lo fp16 pair. The kernel is DMA-bound: ~12.9 MB of x
per core against a ~360 GB/s modeled bus. K=784 is split uniformly
as 7 chunks x 112 partitions (no ragged 16-row tail pass).
"""

import numpy as np

import concourse.bacc as bacc
import concourse.mybir as mybir
import concourse.tile as tile
from concourse.bass_utils import run_bass_kernel_spmd

N_CORES = 8
B = 65536
B_LOCAL = B // N_CORES  # 8192
K = 784                 # input features (28*28)
KP = 112                # K rows per chunk (7 * 112 = 784)
NKC = 7                 # K chunks
M1 = 128                # fc1 out
M2 = 10                 # fc2 out

F32 = mybir.dt.float32
FP16 = mybir.dt.float16

NS = 512                # matmul moving-dim subtile (one PSUM bank)

# Batch-tile schedule: small tiles at the edges so the PE starts early
# (pipeline fill) and finishes right behind the last transfer (drain);
# big tiles in the middle keep the SWDGE generation queue off the
# critical path.
BTS = [256, 256, 512, 1024, 1024, 1024, 1024, 1024, 1024, 512, 256, 256]
assert sum(BTS) == B_LOCAL

_cache = {}


def _build_nc():
    nc = bacc.Bacc("TRN2", target_bir_lowering=False, debug=False,
                   num_devices=N_CORES)

    x_d = nc.dram_tensor("x_t", [K, B_LOCAL], FP16, kind="ExternalInput")
    w1_d = nc.dram_tensor("w1_t", [KP, NKC, M1], FP16, kind="ExternalInput")
    w2_d = nc.dram_tensor("w2_t", [M1, M2], FP16, kind="ExternalInput")
    # f32 pack: col 0 = b1, col 1 rows 0:10 = b2
    bias_d = nc.dram_tensor("biases", [M1, 2], F32, kind="ExternalInput")
    z_d = nc.dram_tensor("z_t", [M2, B_LOCAL], FP16, kind="ExternalOutput")

    x_v = x_d.rearrange("(c p) b -> p c b", p=KP)

    with tile.TileContext(nc) as tc:
        with (
            tc.tile_pool(name="static", bufs=1) as sp,
            tc.tile_pool(name="xp", bufs=4) as xp,
            tc.tile_pool(name="hp", bufs=4) as hp,
            tc.tile_pool(name="zp", bufs=3) as zp,
            tc.tile_pool(name="pp1", bufs=4, space="PSUM") as pp1,
            tc.tile_pool(name="pp2", bufs=2, space="PSUM") as pp2,
        ):
            # Weights ride the SWDGE queue ahead of x tile 0; the small
            # fc2/bias tensors take the (otherwise idle) SP HWDGE path.
            w1 = sp.tile([KP, NKC, M1], FP16, tag="w1")
            nc.gpsimd.dma_start(w1[:], w1_d[:])
            w2 = sp.tile([M1, M2], FP16, tag="w2")
            nc.gpsimd.dma_start(w2[:], w2_d[:])
            bias = sp.tile([M1, 2], F32, tag="biases")
            nc.gpsimd.dma_start(bias[:], bias_d[:])
            b1t = bias[:, 0:1]
            b2t = bias[0:M2, 1:2]

            offs = [sum(BTS[:i]) for i in range(len(BTS))]
            xtiles = [None] * len(BTS)

            def load_bt(i):
                btc = BTS[i]
                xt = xp.tile([KP, NKC, btc], FP16, tag="x")
                nc.gpsimd.dma_start(
                    xt[:], x_v[:, :, offs[i]:offs[i] + btc])
                xtiles[i] = xt

            load_bt(0)
            load_bt(1)
            load_bt(2)

            # Each chunk's fc2 matmul + bias-add is deferred until after
            # the NEXT chunk's fc1 stream, so the PE never sits behind an
            # fc2 that is still waiting on ACT's h output.
            pending = []

            def flush_pending():
                for h_t, zt_t, sl_t, final in pending:
                    ps2 = pp2.tile([M2, sl_t.stop - sl_t.start], F32,
                                   tag="ps2")
                    nc.tensor.matmul(ps2[:], w2[:], h_t[:],
                                     start=True, stop=True)
                    nc.vector.tensor_scalar_add(zt_t[:, sl_t], ps2[:], b2t)
                    if final is not None:
                        nc.gpsimd.dma_start(final[0], zt_t[:])
                pending.clear()

            for bt_i, btc in enumerate(BTS):
                if bt_i + 3 < len(BTS):
                    load_bt(bt_i + 3)
                xt = xtiles[bt_i]
                zt = zp.tile([M2, btc], FP16, tag="z")
                ns = min(NS, btc)
                nchains = btc // ns
                for ns_i in range(nchains):
                    sl = slice(ns_i * ns, (ns_i + 1) * ns)
                    ps1 = pp1.tile([M1, ns], F32, tag="ps1")
                    for c in range(NKC):
                        nc.tensor.matmul(
                            ps1[:], w1[:, c, :], xt[:, c, sl],
                            start=(c == 0), stop=(c == NKC - 1))
                    h = hp.tile([M1, ns], FP16, tag="h")
                    nc.scalar.activation(
                        h[:], ps1[:], mybir.ActivationFunctionType.Relu,
                        bias=b1t)
                    flush_pending()
                    final = None
                    if ns_i == nchains - 1:
                        final = (z_d[:, offs[bt_i]:offs[bt_i] + btc],)
                    pending.append((h, zt, sl, final))
            flush_pending()
    nc.compile()
    return nc


def _fold_weights(conv_w, fc1_w):
    """Fold 3x3 valid cross-correlation + fc1 into one [128, 784] matrix."""
    cw = np.asarray(conv_w, np.float64)
    f1 = np.asarray(fc1_w, np.float64).reshape(M1, 26, 26)
    W = np.zeros((M1, 28, 28), np.float64)
    for di in range(3):
        for dj in range(3):
            W[:, di:di + 26, dj:dj + 26] += cw[di, dj] * f1
    return W.reshape(M1, K).astype(np.float32)


def kernel(x, conv_w, fc1_w, fc1_b, fc2_w, fc2_b):
    if "nc" not in _cache:
        _cache["nc"] = _build_nc()
    nc = _cache["nc"]

    w1t = _fold_weights(conv_w, fc1_w).T.astype(np.float16)  # [784, 128]
    w1t = np.ascontiguousarray(
        w1t.reshape(NKC, KP, M1).transpose(1, 0, 2))  # [112, 7, 128]
    w2t = np.ascontiguousarray(
        np.asarray(fc2_w, np.float16).T)  # [128, 10]
    biases = np.zeros((M1, 2), np.float32)
    biases[:, 0] = np.asarray(fc1_b, np.float32)
    biases[0:M2, 1] = np.asarray(fc2_b, np.float32)
    x = np.asarray(x, np.float32)

    in_maps = []
    for c in range(N_CORES):
        xs = np.ascontiguousarray(
            x[c * B_LOCAL:(c + 1) * B_LOCAL].T.astype(np.float16))
        in_maps.append({
            "x_t": xs, "w1_t": w1t, "w2_t": w2t, "biases": biases,
        })
    res = run_bass_kernel_spmd(nc, in_maps, list(range(N_CORES)))
    outs = [res.results[c]["z_t"].T for c in range(N_CORES)]
    return np.ascontiguousarray(np.concatenate(outs, axis=0), dtype=np.float32)
